# revision 34
# baseline (speedup 1.0000x reference)
"""Trainium2 kernel for: out = tanh(x @ scatter_nd(nonzero_ind, kernel_vector, (20000, 4096)) + bias).

Strategy (8 NeuronCores):
  - Host builds the dense (20000, 4096) fp16 weight matrix from the COO
    triples, pads K to 157*128 = 20096, pre-transposes x to fp16
    xT (20096, 2048) (shared by all cores), and pre-swizzles each
    core's W shard to partition-major [128, 157, 512] so a whole
    K-window loads as ONE DMA of 128 contiguous 20KB lines.
  - Shard units 8-ways: core c owns output columns [c*512, (c+1)*512).
    No K-split => 157 K-tiles (vs 160 for a padded K-quartering), no
    host-side partial summation.
  - Output-stationary windowed accumulation: K in 8 equal windows
    (7x20 + 17 tiles). Each window runs 4 batch quarter-passes of 512
    columns; a pass accumulates in 4 PSUM banks, ping-ponged by pass
    parity so each pass epilogue has a full pass of slack. Window
    partials fold into SBUF fp32 accumulators (even unit chunks on the
    DVE straight from PSUM; odd ones stage via the Activation engine
    then add on Pool, which cannot touch PSUM).
  - W window tiles: 4 resident in rotation (80 KB/partition); all DMA
    issue is one descriptor-gen per window, so the SP sequencer carries
    only the x stream (sequencer descriptor-gen rate, ~0.6us per
    dma_start, is the scarce resource — per-tile W DMAs starve the PE).
    Window 0 is split into 4 sub-DMAs so the first matmul starts ~9us in.
  - The last window preloads ACC back into PSUM (riding right behind
    window 6's folds) and matmuls continue with start=False, so the
    final epilogue reads PSUM directly: fused out = tanh(psum + bias)
    on the Activation engine, fp16 output.
  - PE warmup: dummy matmuls on a memset tile keep the PE busy through
    the initial DMA fill so its p-state ramp (-> 2.4 GHz) completes
    before real work starts.
  - Host only concatenates/transposes the 8 shards.
"""

import numpy as np

P = 128
B, K, U = 2048, 20000, 4096
KT = 157                 # ceil(20000 / 128)
KPAD = KT * P            # 20096
U_SH = U // 8            # 512 units per core
UC = U_SH // P           # 4 unit chunks per core
QB = 512                 # batch block width (PSUM bank free size)
NB = B // QB             # 4 batch quarters
WINS = [6, 7, 8, 9, 11, 13, 15, 18, 21, 25, 24]  # K-window sizes (sum = 157)
W_RES = 4                # W window tiles resident (rotation, windows 1+)
W0SUB = 2                # k-tiles per window-0 sub-tile
N_WARM = 14              # warmup matmuls (free dim 256) covering DMA fill
WARM_F = 256

TRACE = False            # set by test harness for profiled runs
LAST_RESULT = None       # BassKernelResults of the last run (for the harness)

_NC_CACHE = {}


def _build_nc():
    from concourse import bacc
    import concourse.mybir as mybir
    import concourse.tile as tile

    f32 = mybir.dt.float32
    f16 = mybir.dt.float16
    Tanh = mybir.ActivationFunctionType.Tanh
    add_op = mybir.AluOpType.add

    assert sum(WINS) == KT
    NW = len(WINS)
    offs = [sum(WINS[:i]) for i in range(NW)]

    nc = bacc.Bacc("TRN2", target_bir_lowering=False, debug=False)
    xt_d = nc.dram_tensor("xt_sh", [KPAD, B], f16, kind="ExternalInput").ap()
    w_d = nc.dram_tensor("w_sh", [P, KT * U_SH], f16, kind="ExternalInput").ap()
    b_d = nc.dram_tensor("b_sh", [UC, P, 1], f32, kind="ExternalInput").ap()
    o_d = nc.dram_tensor("out_p", [U_SH, B], f16, kind="ExternalOutput").ap()

    with tile.TileContext(nc) as tc:
        with (
            tc.tile_pool(name="wpanel", bufs=1) as wpool,
            tc.tile_pool(name="xquart", bufs=16) as xqpool,
            tc.tile_pool(name="accpool", bufs=1) as apool,
            tc.tile_pool(name="bias", bufs=1) as bpool,
            tc.tile_pool(name="ostage", bufs=4) as opool,
            tc.tile_pool(name="fstage", bufs=4) as spool,
            tc.tile_pool(name="warm", bufs=1) as warmpool,
            tc.tile_pool(name="mpsum", bufs=1, space="PSUM") as mpsum,
        ):
            # ---- epilogue helpers (engine-split; Pool can't touch PSUM) --
            def fold(b, uc, psum, first):
                if uc % 2 == 0:
                    if first:
                        nc.vector.tensor_copy(acc[b][uc][:], psum[:])
                    else:
                        nc.vector.tensor_tensor(
                            acc[b][uc][:], acc[b][uc][:], psum[:], add_op
                        )
                else:
                    if first:
                        nc.scalar.copy(acc[b][uc][:], psum[:])
                    else:
                        st = spool.tile([P, QB], f32, tag="st", name=f"st{b}_{uc}")
                        nc.scalar.copy(st[:], psum[:])
                        nc.gpsimd.tensor_tensor(
                            acc[b][uc][:], acc[b][uc][:], st[:], add_op
                        )

            def preload_copy(psum, b, uc):
                if uc % 2 == 0:
                    nc.vector.tensor_copy(psum[:], acc[b][uc][:])
                else:
                    nc.scalar.copy(psum[:], acc[b][uc][:])

            def psum_quarter(q, label):
                return [
                    mpsum.tile(
                        [P, QB], f32,
                        tag=f"ps{4 * (q % 2) + uc}",
                        name=f"ps_{label}_{uc}",
                    )
                    for uc in range(UC)
                ]

            # PE warmup (parks in bank ps7; real work reaches it late).
            warm = warmpool.tile([P, WARM_F], f16, tag="warm", name="warm")
            nc.vector.memset(warm[:], 0.0)
            warm_ps = mpsum.tile([P, WARM_F], f32, tag="ps7", name="warm_ps")
            for _ in range(N_WARM):
                nc.tensor.matmul(
                    warm_ps[:], warm[:, :P], warm[:], start=True, stop=True
                )

            # W window tiles. Window 0 lives in small independent sub-tiles
            # (tile-granular dependencies: one big tile would stall the
            # first matmul until the whole window landed). Windows 1+ use
            # 4 rotating max-size tiles; each window's data arrives as 4
            # sub-DMAs paced across the previous window's quarters.
            WMAX = max(WINS)

            def w_sub_dma(wi, t, s0, n):
                c0 = (offs[wi] + s0) * U_SH
                nc.scalar.dma_start(
                    t[:, s0 * U_SH:(s0 + n) * U_SH],
                    w_d[:, c0:c0 + n * U_SH],
                )

            wwin0 = []
            for s in range(0, WINS[0], W0SUB):
                st = wpool.tile(
                    [P, W0SUB * U_SH], f16, tag=f"w0s{s}", name=f"w0s{s}"
                )
                nc.scalar.dma_start(
                    st[:],
                    w_d[:, (offs[0] + s) * U_SH:(offs[0] + s + W0SUB) * U_SH],
                )
                wwin0.append(st)

            def w_tile(wi):
                return wpool.tile(
                    [P, WMAX * U_SH], f16, tag=f"w{(wi - 1) % W_RES}",
                    name=f"ww{wi}",
                )

            # Per-partition bias chunks (psum partition dim = units).
            bias_t = []
            for uc in range(UC):
                bt = bpool.tile([P, 1], f32, tag=f"b{uc}", name=f"b{uc}")
                nc.scalar.dma_start(bt[:], b_d[uc])
                bias_t.append(bt)

            # SBUF fp32 accumulators, one per (batch quarter, unit chunk).
            acc = [
                [
                    apool.tile([P, QB], f32, tag=f"acc{b}_{uc}", name=f"acc{b}_{uc}")
                    for uc in range(UC)
                ]
                for b in range(NB)
            ]

            preloaded = {}
            wnext = None

            for wi in range(NW):
                L = WINS[wi]
                last_win = wi == NW - 1
                wt = wnext
                wnext = None
                for q in range(NB):
                    # Pace the next window's W in as 4 quarter-sized
                    # sub-DMAs spread across this window.
                    if wi + 1 < NW:
                        nl = WINS[wi + 1]
                        chunk = (nl + NB - 1) // NB
                        s0 = q * chunk
                        if q == 0:
                            wnext = w_tile(wi + 1)
                        if s0 < nl:
                            w_sub_dma(wi + 1, wnext, s0, min(chunk, nl - s0))
                    if last_win:
                        if q in preloaded:
                            psums = preloaded.pop(q)
                        else:
                            psums = psum_quarter(q, f"{wi}_{q}")
                            for uc in range(UC):
                                preload_copy(psums[uc], q, uc)
                    else:
                        psums = psum_quarter(q, f"{wi}_{q}")
                    for j in range(L):
                        kt = offs[wi] + j
                        xt = xqpool.tile([P, QB], f16, tag="xq", name=f"x{kt}_{q}")
                        nc.sync.dma_start(
                            xt[:],
                            xt_d[kt * P:(kt + 1) * P, q * QB:(q + 1) * QB],
                        )
                        if wi == 0:
                            wap = wwin0[j // W0SUB]
                            jo = (j % W0SUB) * U_SH
                        else:
                            wap = wt
                            jo = j * U_SH
                        for uc in range(UC):
                            nc.tensor.matmul(
                                psums[uc][:],
                                wap[:, jo + uc * P:jo + (uc + 1) * P],
                                xt[:],
                                start=(j == 0 and not last_win),
                                stop=(j == L - 1),
                                skip_group_check=last_win,
                            )
                    if last_win:
                        # Fused epilogue: tanh(psum + bias) -> fp16,
                        # straight from PSUM on the Activation engine.
                        # The final quarter splits each act/DMA in half and
                        # puts the DMAs on the then-idle SP sequencer, so
                        # the tail chases the last matmuls at 256-column
                        # granularity.
                        splits = 2 if q == NB - 1 else 1
                        for uc in range(UC):
                            ot = opool.tile([P, QB], f16, tag="ot", name=f"o{q}_{uc}")
                            hw = QB // splits
                            for hh in range(splits):
                                nc.scalar.activation(
                                    ot[:, hh * hw:(hh + 1) * hw],
                                    psums[uc][:, hh * hw:(hh + 1) * hw],
                                    Tanh, bias=bias_t[uc][:], scale=1.0,
                                )
                                oeng = nc.sync if q == NB - 1 else nc.scalar
                                oeng.dma_start(
                                    o_d[uc * P:(uc + 1) * P,
                                        q * QB + hh * hw:q * QB + (hh + 1) * hw],
                                    ot[:, hh * hw:(hh + 1) * hw],
                                )
                    else:
                        for uc in range(UC):
                            fold(q, uc, psums[uc], wi == 0)
                        if wi == NW - 2 and q >= 2:
                            # Preload the last window's same-parity quarter
                            # right behind these folds.
                            lq = q - 2
                            lps = psum_quarter(lq, f"{NW - 1}_{lq}")
                            for uc in range(UC):
                                preload_copy(lps[uc], lq, uc)
                            preloaded[lq] = lps


    nc.compile()
    return nc


def _get_nc(key=("v6",)):
    if key not in _NC_CACHE:
        _NC_CACHE[key] = _build_nc()
    return _NC_CACHE[key]


def kernel(x, kernel_vector, bias, nonzero_ind):
    global LAST_RESULT
    from concourse.bass_utils import run_bass_kernel_spmd

    x = np.asarray(x, dtype=np.float32)
    kernel_vector = np.asarray(kernel_vector, dtype=np.float32)
    bias = np.asarray(bias, dtype=np.float32)
    nonzero_ind = np.asarray(nonzero_ind)

    nc = _get_nc()

    # Host scatter: dense fp16 weights, K padded to 157*128.
    rows = nonzero_ind[:, 0].astype(np.int64)
    cols = nonzero_ind[:, 1].astype(np.int64)
    w_full = np.zeros(KPAD * U, np.float32)
    np.add.at(w_full, rows * U + cols, kernel_vector)
    w_full = w_full.reshape(KPAD, U).astype(np.float16)

    # Shared transposed x, fp16, K-padded.
    xt = np.zeros((KPAD, B), np.float16)
    xt[:K] = x.astype(np.float16).T

    in_maps = []
    for c in range(8):
        # Swizzle the W shard partition-major: [128, 157, 512].
        wsh = w_full[:, c * U_SH:(c + 1) * U_SH].reshape(KT, P, U_SH)
        wsh = np.ascontiguousarray(wsh.transpose(1, 0, 2)).reshape(P, KT * U_SH)
        in_maps.append({
            "xt_sh": xt,
            "w_sh": wsh,
            "b_sh": np.ascontiguousarray(
                bias[c * U_SH:(c + 1) * U_SH].reshape(UC, P, 1)
            ),
        })

    kwargs = {}
    if TRACE:
        kwargs = dict(trace=True, trace_cores=list(range(8)))
    res = run_bass_kernel_spmd(nc, in_maps, core_ids=list(range(8)), **kwargs)
    LAST_RESULT = res

    # Device already applied bias + tanh; just assemble (out is [U, B]).
    out_t = np.concatenate(
        [res.results[c]["out_p"] for c in range(8)], axis=0
    )
    return out_t.T.astype(np.float32)


# revision 35
# speedup vs baseline: 1.0030x; 1.0030x over previous
"""Trainium2 kernel for: out = tanh(x @ scatter_nd(nonzero_ind, kernel_vector, (20000, 4096)) + bias).

Strategy (8 NeuronCores):
  - Host builds the dense (20000, 4096) fp16 weight matrix from the COO
    triples, pads K to 157*128 = 20096, pre-transposes x to fp16
    xT (20096, 2048) (shared by all cores), and pre-swizzles each
    core's W shard to partition-major [128, 157, 512] so a whole
    K-window loads as ONE DMA of 128 contiguous 20KB lines.
  - Shard units 8-ways: core c owns output columns [c*512, (c+1)*512).
    No K-split => 157 K-tiles (vs 160 for a padded K-quartering), no
    host-side partial summation.
  - Output-stationary windowed accumulation: K in 8 equal windows
    (7x20 + 17 tiles). Each window runs 4 batch quarter-passes of 512
    columns; a pass accumulates in 4 PSUM banks, ping-ponged by pass
    parity so each pass epilogue has a full pass of slack. Window
    partials fold into SBUF fp32 accumulators (even unit chunks on the
    DVE straight from PSUM; odd ones stage via the Activation engine
    then add on Pool, which cannot touch PSUM).
  - W window tiles: 4 resident in rotation (80 KB/partition); all DMA
    issue is one descriptor-gen per window, so the SP sequencer carries
    only the x stream (sequencer descriptor-gen rate, ~0.6us per
    dma_start, is the scarce resource — per-tile W DMAs starve the PE).
    Window 0 is split into 4 sub-DMAs so the first matmul starts ~9us in.
  - The last window preloads ACC back into PSUM (riding right behind
    window 6's folds) and matmuls continue with start=False, so the
    final epilogue reads PSUM directly: fused out = tanh(psum + bias)
    on the Activation engine, fp16 output.
  - PE warmup: dummy matmuls on a memset tile keep the PE busy through
    the initial DMA fill so its p-state ramp (-> 2.4 GHz) completes
    before real work starts.
  - Host only concatenates/transposes the 8 shards.
"""

import numpy as np

P = 128
B, K, U = 2048, 20000, 4096
KT = 157                 # ceil(20000 / 128)
KPAD = KT * P            # 20096
U_SH = U // 8            # 512 units per core
UC = U_SH // P           # 4 unit chunks per core
QB = 512                 # batch block width (PSUM bank free size)
NB = B // QB             # 4 batch quarters
WINS = [6, 7, 8, 9, 11, 13, 15, 18, 21, 25, 24]  # K-window sizes (sum = 157)
W_RES = 4                # W window tiles resident (rotation, windows 1+)
W0SUB = 2                # k-tiles per window-0 sub-tile
N_WARM = 14              # warmup matmuls (free dim 256) covering DMA fill
WARM_F = 256

TRACE = False            # set by test harness for profiled runs
LAST_RESULT = None       # BassKernelResults of the last run (for the harness)

_NC_CACHE = {}


def _build_nc():
    from concourse import bacc
    import concourse.mybir as mybir
    import concourse.tile as tile

    f32 = mybir.dt.float32
    f16 = mybir.dt.float16
    Tanh = mybir.ActivationFunctionType.Tanh
    add_op = mybir.AluOpType.add

    assert sum(WINS) == KT
    NW = len(WINS)
    offs = [sum(WINS[:i]) for i in range(NW)]

    nc = bacc.Bacc("TRN2", target_bir_lowering=False, debug=False)
    xt_d = nc.dram_tensor("xt_sh", [KPAD, B], f16, kind="ExternalInput").ap()
    w_d = nc.dram_tensor("w_sh", [P, KT * U_SH], f16, kind="ExternalInput").ap()
    b_d = nc.dram_tensor("b_sh", [UC, P, 1], f32, kind="ExternalInput").ap()
    o_d = nc.dram_tensor("out_p", [U_SH, B], f16, kind="ExternalOutput").ap()

    with tile.TileContext(nc) as tc:
        with (
            tc.tile_pool(name="wpanel", bufs=1) as wpool,
            tc.tile_pool(name="xquart", bufs=16) as xqpool,
            tc.tile_pool(name="accpool", bufs=1) as apool,
            tc.tile_pool(name="bias", bufs=1) as bpool,
            tc.tile_pool(name="ostage", bufs=4) as opool,
            tc.tile_pool(name="fstage", bufs=4) as spool,
            tc.tile_pool(name="warm", bufs=1) as warmpool,
            tc.tile_pool(name="mpsum", bufs=1, space="PSUM") as mpsum,
        ):
            # ---- epilogue helpers (engine-split; Pool can't touch PSUM) --
            def fold(b, uc, psum, first):
                if uc % 2 == 0:
                    if first:
                        nc.vector.tensor_copy(acc[b][uc][:], psum[:])
                    else:
                        nc.vector.tensor_tensor(
                            acc[b][uc][:], acc[b][uc][:], psum[:], add_op
                        )
                else:
                    if first:
                        nc.scalar.copy(acc[b][uc][:], psum[:])
                    else:
                        st = spool.tile([P, QB], f32, tag="st", name=f"st{b}_{uc}")
                        nc.scalar.copy(st[:], psum[:])
                        nc.gpsimd.tensor_tensor(
                            acc[b][uc][:], acc[b][uc][:], st[:], add_op
                        )

            def preload_copy(psum, b, uc):
                if uc % 2 == 0:
                    nc.vector.tensor_copy(psum[:], acc[b][uc][:])
                else:
                    nc.scalar.copy(psum[:], acc[b][uc][:])

            def psum_quarter(q, label):
                return [
                    mpsum.tile(
                        [P, QB], f32,
                        tag=f"ps{4 * (q % 2) + uc}",
                        name=f"ps_{label}_{uc}",
                    )
                    for uc in range(UC)
                ]

            # PE warmup (parks in bank ps7; real work reaches it late).
            warm = warmpool.tile([P, WARM_F], f16, tag="warm", name="warm")
            nc.vector.memset(warm[:], 0.0)
            warm_ps = mpsum.tile([P, WARM_F], f32, tag="ps7", name="warm_ps")
            for _ in range(N_WARM):
                nc.tensor.matmul(
                    warm_ps[:], warm[:, :P], warm[:], start=True, stop=True
                )

            # W window tiles. Window 0 lives in small independent sub-tiles
            # (tile-granular dependencies: one big tile would stall the
            # first matmul until the whole window landed). Windows 1+ use
            # 4 rotating max-size tiles; each window's data arrives as 4
            # sub-DMAs paced across the previous window's quarters.
            WMAX = max(WINS)

            def w_sub_dma(wi, t, s0, n):
                c0 = (offs[wi] + s0) * U_SH
                nc.scalar.dma_start(
                    t[:, s0 * U_SH:(s0 + n) * U_SH],
                    w_d[:, c0:c0 + n * U_SH],
                )

            wwin0 = []
            for s in range(0, WINS[0], W0SUB):
                st = wpool.tile(
                    [P, W0SUB * U_SH], f16, tag=f"w0s{s}", name=f"w0s{s}"
                )
                nc.scalar.dma_start(
                    st[:],
                    w_d[:, (offs[0] + s) * U_SH:(offs[0] + s + W0SUB) * U_SH],
                )
                wwin0.append(st)

            def w_tile(wi):
                return wpool.tile(
                    [P, WMAX * U_SH], f16, tag=f"w{(wi - 1) % W_RES}",
                    name=f"ww{wi}",
                )

            # Per-partition bias chunks (psum partition dim = units).
            bias_t = []
            for uc in range(UC):
                bt = bpool.tile([P, 1], f32, tag=f"b{uc}", name=f"b{uc}")
                nc.scalar.dma_start(bt[:], b_d[uc])
                bias_t.append(bt)

            # SBUF fp32 accumulators, one per (batch quarter, unit chunk).
            acc = [
                [
                    apool.tile([P, QB], f32, tag=f"acc{b}_{uc}", name=f"acc{b}_{uc}")
                    for uc in range(UC)
                ]
                for b in range(NB)
            ]

            preloaded = {}
            wnext = None

            for wi in range(NW):
                L = WINS[wi]
                last_win = wi == NW - 1
                wt = wnext
                wnext = None
                for q in range(NB):
                    # Pace the next window's W in as 4 quarter-sized
                    # sub-DMAs spread across this window.
                    if wi + 1 < NW:
                        nl = WINS[wi + 1]
                        chunk = (nl + NB - 1) // NB
                        s0 = q * chunk
                        if q == 0:
                            wnext = w_tile(wi + 1)
                        if s0 < nl:
                            w_sub_dma(wi + 1, wnext, s0, min(chunk, nl - s0))
                    if last_win:
                        if q in preloaded:
                            psums = preloaded.pop(q)
                        else:
                            psums = psum_quarter(q, f"{wi}_{q}")
                            for uc in range(UC):
                                preload_copy(psums[uc], q, uc)
                    else:
                        psums = psum_quarter(q, f"{wi}_{q}")
                    for j in range(L):
                        kt = offs[wi] + j
                        xt = xqpool.tile([P, QB], f16, tag="xq", name=f"x{kt}_{q}")
                        nc.sync.dma_start(
                            xt[:],
                            xt_d[kt * P:(kt + 1) * P, q * QB:(q + 1) * QB],
                        )
                        if wi == 0:
                            wap = wwin0[j // W0SUB]
                            jo = (j % W0SUB) * U_SH
                        else:
                            wap = wt
                            jo = j * U_SH
                        for uc in range(UC):
                            nc.tensor.matmul(
                                psums[uc][:],
                                wap[:, jo + uc * P:jo + (uc + 1) * P],
                                xt[:],
                                start=(j == 0 and not last_win),
                                stop=(j == L - 1),
                                skip_group_check=last_win,
                            )
                    if last_win:
                        # Fused epilogue: tanh(psum + bias) -> fp16,
                        # straight from PSUM on the Activation engine.
                        # The final quarter's output DMAs ride the then-idle
                        # SP sequencer so the Activation queue can chase the
                        # last matmuls with its activations.
                        for uc in range(UC):
                            ot = opool.tile([P, QB], f16, tag="ot", name=f"o{q}_{uc}")
                            nc.scalar.activation(
                                ot[:], psums[uc][:], Tanh,
                                bias=bias_t[uc][:], scale=1.0,
                            )
                            oeng = nc.sync if q == NB - 1 else nc.scalar
                            oeng.dma_start(
                                o_d[uc * P:(uc + 1) * P, q * QB:(q + 1) * QB],
                                ot[:],
                            )
                    else:
                        for uc in range(UC):
                            fold(q, uc, psums[uc], wi == 0)
                        if wi == NW - 2 and q >= 2:
                            # Preload the last window's same-parity quarter
                            # right behind these folds.
                            lq = q - 2
                            lps = psum_quarter(lq, f"{NW - 1}_{lq}")
                            for uc in range(UC):
                                preload_copy(lps[uc], lq, uc)
                            preloaded[lq] = lps


    nc.compile()
    return nc


def _get_nc(key=("v6",)):
    if key not in _NC_CACHE:
        _NC_CACHE[key] = _build_nc()
    return _NC_CACHE[key]


def kernel(x, kernel_vector, bias, nonzero_ind):
    global LAST_RESULT
    from concourse.bass_utils import run_bass_kernel_spmd

    x = np.asarray(x, dtype=np.float32)
    kernel_vector = np.asarray(kernel_vector, dtype=np.float32)
    bias = np.asarray(bias, dtype=np.float32)
    nonzero_ind = np.asarray(nonzero_ind)

    nc = _get_nc()

    # Host scatter: dense fp16 weights, K padded to 157*128.
    rows = nonzero_ind[:, 0].astype(np.int64)
    cols = nonzero_ind[:, 1].astype(np.int64)
    w_full = np.zeros(KPAD * U, np.float32)
    np.add.at(w_full, rows * U + cols, kernel_vector)
    w_full = w_full.reshape(KPAD, U).astype(np.float16)

    # Shared transposed x, fp16, K-padded.
    xt = np.zeros((KPAD, B), np.float16)
    xt[:K] = x.astype(np.float16).T

    in_maps = []
    for c in range(8):
        # Swizzle the W shard partition-major: [128, 157, 512].
        wsh = w_full[:, c * U_SH:(c + 1) * U_SH].reshape(KT, P, U_SH)
        wsh = np.ascontiguousarray(wsh.transpose(1, 0, 2)).reshape(P, KT * U_SH)
        in_maps.append({
            "xt_sh": xt,
            "w_sh": wsh,
            "b_sh": np.ascontiguousarray(
                bias[c * U_SH:(c + 1) * U_SH].reshape(UC, P, 1)
            ),
        })

    kwargs = {}
    if TRACE:
        kwargs = dict(trace=True, trace_cores=list(range(8)))
    res = run_bass_kernel_spmd(nc, in_maps, core_ids=list(range(8)), **kwargs)
    LAST_RESULT = res

    # Device already applied bias + tanh; just assemble (out is [U, B]).
    out_t = np.concatenate(
        [res.results[c]["out_p"] for c in range(8)], axis=0
    )
    return out_t.T.astype(np.float32)
